# revision 1
# baseline (speedup 1.0000x reference)
"""Trainium2 Bass kernel for nn_RandomResizedCrop — bf16, rebalanced engines.

Same math as the fp32 baseline (see kernel.py docstring): the 7/8 resample
is periodic with period 32 inside each fp32 octave; 64 output tiles of
524288 elems are processed as 8 slots x 8 cores.  This version:

  * stores windows / t-streams / weights / output in bf16 (tolerance gate is
    2e-2 normalized; measured bf16 error ~9e-3) -> halves HBM traffic and
    doubles DVE throughput (2x packed mode),
  * keeps all TT/STT work on the vector engine (gpsimd is ~2.6 cyc/elem),
  * merges the per-phase op loops into 4D-AP instructions where the tap
    offset is affine in the phase index (E32[b] = b-1 for b in [1,8)),
  * gives every slot dedicated SBUF tiles so the Tile scheduler can stream
    all input DMAs back-to-back and overlap compute freely.
"""

import numpy as np

N = 33554432
CL = 29360128
SP = 1000000
TILE = 524288          # outputs per tile
FO = 4096              # outputs per partition
NPART = 128
A2T = FO // 32         # 128 phase blocks per partition
WROWS = 32             # transposed window rows: elem = cropped[start + 28*a + r]
W = WROWS * 128        # window floats per partition (phase-planar layout)

SLOT_TILES = [
    [0, 1, 2, 3, 5, 6, 7, 8],                # U1 (exact + oct-21 patterns)
    [10, 11, 12, 13, 14, 15, 16, 17],        # static-A oct-22
    [19, 20, 21, 22, 23, 24, 25, 26],        # static-A oct-23
    [4, 9, 18, 27, 28, 29, 30, 31],          # U1 (impure + oct-23 leftovers)
    [36, 32, 33, 34, 35, 61, 62, 63],        # U2 streamed integer select
    [37, 38, 39, 40, 41, 42, 43, 44],        # static-B oct-24
    [45, 46, 47, 48, 49, 50, 51, 52],        # static-B oct-24
    [53, 54, 55, 56, 57, 58, 59, 60],        # static-B oct-24
]
SLOT_KIND = ["U1", "A", "A", "U1", "U2", "B", "B", "B"]

E32 = (7 * np.arange(32)) // 8

_STATE = {}


def _run_plan(off):
    """Greedy segmentation of phases [0,32) into maximal affine runs.
    Returns list of (b0, L, src0, dstep): off[b0+r] == src0 + r*dstep."""
    plans, b = [], 0
    off = [int(x) for x in off]
    while b < 32:
        if b == 31:
            plans.append((b, 1, off[b], 0)); break
        d = off[b + 1] - off[b]
        L = 2
        while b + L < 32 and off[b + L] == off[b] + L * d:
            L += 1
        plans.append((b, L, off[b], d)); b += L
    return plans


def _j_merge(off, wv=None):
    """Check stride-8 (4-way) then stride-16 (2-way) phase merging."""
    off = [int(x) for x in off]
    for jb, nj in ((8, 4), (16, 2)):
        sstep = 7 * (jb // 8)
        ok = all(off[b + jb * j] == off[b] + sstep * j
                 for b in range(jb) for j in range(nj))
        if ok and wv is not None:
            ok = all(np.array_equal(wv[:, :, b + jb * j], wv[:, :, b])
                     for b in range(jb) for j in range(1, nj))
        if ok:
            return jb, nj, sstep
    return None


def _repack_j4(arr):
    """[.., 32, A2T] phase-major -> [.., 8, 4, A2T] op-major (b, j) order."""
    v = arr.reshape(arr.shape[:-1] + (32, A2T))
    v = v.reshape(v.shape[:-2] + (4, 8, A2T))   # [j, b, a]
    v = np.swapaxes(v, -3, -2)                  # [b, j, a]
    return np.ascontiguousarray(v).reshape(arr.shape)


def _build_tables():
    i = np.arange(N, dtype=np.int64)
    idx = (np.float32(0.875) * i.astype(np.float32)).astype(np.float32)
    idx[-1] = np.float32(CL)
    a2 = i // 32
    b = i % 32
    T = idx.astype(np.float64) - (28 * a2 + E32[b])
    Tt = T.reshape(64, NPART, A2T, 32)

    starts, offs, wvec, tstream = {}, {}, {}, {}
    for s, tiles in enumerate(SLOT_TILES):
        kind = SLOT_KIND[s]
        st = np.zeros((8, NPART), dtype=np.int64)
        if kind in ("A", "B"):
            off_ref = None
            wv = np.zeros((8, NPART, 32), dtype=np.float32)
            for c, tl in enumerate(tiles):
                Tp = Tt[tl]
                base = np.floor(Tp.min(axis=(1, 2))).astype(np.int64)
                trel = Tp - base[:, None, None]
                cmin = trel.min(axis=1)
                assert np.array_equal(cmin, trel.max(axis=1))
                off = np.floor(cmin).astype(np.int64)
                wv[c] = (cmin - off).astype(np.float32)
                assert np.all(off == off[0:1, :])
                if off_ref is None:
                    off_ref = off[0]
                assert np.array_equal(off[0], off_ref)
                if kind == "B":
                    assert np.all(wv[c] == 0)
                a2g = (tl * TILE + np.arange(NPART) * FO) // 32
                st[c] = 28 * a2g + base
            offs[s] = (E32 + off_ref).astype(np.int64)
            assert offs[s].max() <= WROWS - 2, offs[s].max()
            wvec[s] = wv
        elif kind == "U1":
            ts = np.zeros((8, NPART, 32, A2T), dtype=np.float32)
            for c, tl in enumerate(tiles):
                Tp = Tt[tl]
                base = np.floor(Tp.min(axis=(1, 2))).astype(np.int64)
                trel = Tp - base[:, None, None]
                t32 = trel.astype(np.float32)
                assert np.all(t32.astype(np.float64) == trel)
                assert 0 <= t32.min() and t32.max() <= 1.0
                ts[c] = np.transpose(t32, (0, 2, 1))
                a2g = (tl * TILE + np.arange(NPART) * FO) // 32
                st[c] = 28 * a2g + base
            tstream[s] = _repack_j4(ts.reshape(8, NPART, 32 * A2T))
        else:  # U2: integer t in {0..3} -> three uint8 level masks
            mk = np.zeros((8, 3, NPART, 32, A2T), dtype=np.uint8)
            for c, tl in enumerate(tiles):
                Tp = Tt[tl]
                base = np.floor(Tp.min(axis=(1, 2))).astype(np.int64)
                trel = Tp - base[:, None, None]
                assert np.all(trel == np.round(trel)) and trel.max() <= 3.0
                ti = np.transpose(trel.astype(np.int64), (0, 2, 1))  # [128,32,A2T]
                for k in (1, 2, 3):
                    mk[c, k - 1] = (ti >= k).astype(np.uint8)
                a2g = (tl * TILE + np.arange(NPART) * FO) // 32
                st[c] = 28 * a2g + base
            tstream[s] = _repack_j4(mk.reshape(8, 3, NPART, 32 * A2T))
        starts[s] = st
    return starts, offs, wvec, tstream


def _build_nc(offs, wvec_chk=None, reps=1, mode='full'):
    import bass_rust
    import concourse.bacc as bacc
    import concourse.mybir as mybir
    from concourse.tile import TileContext

    bf16 = mybir.dt.bfloat16
    u8 = mybir.dt.uint8
    Alu = mybir.AluOpType

    nc = bacc.Bacc("TRN2", target_bir_lowering=False)
    win_t = None  # created after rmaxs known (per-slot sizes)
    t_t = {s: nc.dram_tensor(f"t{s}", [NPART, 32 * A2T], bf16, kind="ExternalInput")
           for s in (0, 3)}
    m_t = [nc.dram_tensor(f"m4_{k}", [NPART, 32 * A2T], u8, kind="ExternalInput")
           for k in (1, 2, 3)]
    wv_t = {s: nc.dram_tensor(f"wv{s}", [NPART, 32], bf16, kind="ExternalInput")
            for s in (1, 2)}
    out_t = nc.dram_tensor("out", [8, NPART, FO], bf16, kind="ExternalOutput")

    def apn(base, off, *dims):
        """AP with free dims [(stride, count), ...] at element offset off."""
        a = base.copy()
        part = list(a.ap[0])
        a.ap = bass_rust.VecI64Pair([part] + [list(d) for d in dims])
        a.offset = a.offset + off
        return a

    # window rows actually read per slot:
    #   B slots only read odd rows 1..29 -> host packs them as 15 rows
    #   U1 slots read dw rows <= 27 -> 29 window rows suffice
    #   U2 reads rows <= 30 -> 31 rows
    rmaxs = {}
    for s in range(8):
        kind = SLOT_KIND[s]
        if kind == "B":
            used = sorted({src0 + r * d for (_, L, src0, d)
                           in _run_plan(offs[s]) for r in range(L)})
            assert used == list(range(1, 30, 2)), (s, used)
            rmaxs[s] = 15
        elif kind == "U1":
            rmaxs[s] = 29
        elif kind == "U2":
            rmaxs[s] = 31
        else:
            rmaxs[s] = WROWS
    win_t = [nc.dram_tensor(f"win{s}", [NPART, rmaxs[s] * A2T], bf16,
                            kind="ExternalInput") for s in range(8)]

    ONLY = mode[4:] if mode.startswith('only') else None

    def on(s):
        return ONLY is None or SLOT_KIND[s] == ONLY

    from contextlib import nullcontext
    with TileContext(nc) as tc:
        with tc.tile_pool(name="p", bufs=1) as pool, \
             (tc.For_i(0, reps, 1) if reps > 1 else nullcontext()):
            wt, ot, dwt, tt, wvt, wvxt, mts = {}, {}, {}, {}, {}, {}, []
            for s in range(8):
                wt[s] = pool.tile([NPART, rmaxs[s] * A2T], bf16,
                                  tag=f"win{s}", name=f"win{s}")
                ot[s] = pool.tile([NPART, FO], bf16, tag=f"out{s}",
                                  name=f"ot{s}")
            for s in (0, 3):
                tt[s] = pool.tile([NPART, 32 * A2T], bf16, tag=f"t{s}",
                                  name=f"tt{s}")
            for k in range(3):
                mts.append(pool.tile([NPART, 32 * A2T], u8, tag=f"m{k}",
                                     name=f"mt{k}"))
            for s in (1, 2):
                wvt[s] = pool.tile([NPART, 32], bf16, tag=f"wv{s}",
                                   name=f"wvt{s}")

            do_compute = mode != 'dma'

            # --- input DMAs in consumption order (sync ring is FIFO) ---
            def dma_win(s):
                if mode != 'compute':
                    nc.sync.dma_start(wt[s][:], win_t[s][:])
                else:
                    nc.gpsimd.memset(wt[s][:, :1], 0.0)

            if on(0):
                dma_win(0)
                if do_compute:
                    nc.sync.dma_start(tt[0][:], t_t[0][:])
            if on(1) and do_compute:
                nc.sync.dma_start(wvt[1][:], wv_t[1][:])
                nc.sync.dma_start(wvt[2][:], wv_t[2][:])
            if on(1):
                dma_win(1)
                dma_win(2)
            if on(3):
                dma_win(3)
                if do_compute:
                    nc.sync.dma_start(tt[3][:], t_t[3][:])
            if on(4):
                dma_win(4)
                if do_compute:
                    for k in range(3):
                        nc.sync.dma_start(mts[k][:], m_t[k][:])
            for s in (5, 6, 7):
                if on(s):
                    dma_win(s)

            # --- compute per slot ---
            for s in range(8):
                kind = SLOT_KIND[s]
                if not on(s):
                    continue

                def osrc(b0, nb):
                    return apn(ot[s][:], b0 * A2T,
                               (A2T, nb), (8 * A2T, 4), (1, A2T))

                def dsrc(base_t, e0, nb, rstep=7):
                    # rows e0 + (b - b0) + rstep*j
                    return apn(base_t[:], e0 * A2T,
                               (A2T, nb), (rstep * A2T, 4), (1, A2T))

                if not do_compute:
                    nc.gpsimd.memset(ot[s][:, :1], 0.0)
                elif kind in ("U1", "A"):
                    # dw = w[r+1] - w[r]  (A-slot dw on gpsimd to unload DVE)
                    ndw = rmaxs[s] - 1
                    dwt[s] = pool.tile([NPART, ndw * A2T], bf16,
                                       tag=f"dw{s}", name=f"dw{s}")
                    dw_eng = nc.gpsimd if kind == "A" else nc.vector
                    dw_eng.tensor_tensor(dwt[s][:], wt[s][:, A2T:],
                                         wt[s][:, :ndw * A2T],
                                         Alu.subtract)
                    if kind == "A":
                        # fused (dw * wv[b]) + win via STT b-loop on gpsimd
                        jm = _j_merge(offs[s], wvec_chk[s])
                        assert jm is not None, (s, jm)
                        jb, nj, sstep = jm
                        for b in range(jb):
                            o = int(offs[s][b])
                            nc.vector.scalar_tensor_tensor(
                                apn(ot[s][:], b * A2T,
                                    (jb * A2T, nj), (1, A2T)),
                                apn(dwt[s][:], o * A2T,
                                    (sstep * A2T, nj), (1, A2T)),
                                wvt[s][:, b:b + 1],
                                apn(wt[s][:], o * A2T,
                                    (sstep * A2T, nj), (1, A2T)),
                                Alu.mult, Alu.add)
                        if mode != 'compute':
                            nc.scalar.dma_start(out_t[s], ot[s][:])
                        continue

                    def tsrc(b0, nb):
                        return apn(tt[s][:], b0 * 4 * A2T,
                                   (4 * A2T, nb), (A2T, 4), (1, A2T))

                    nc.vector.tensor_tensor(
                        osrc(0, 1), tsrc(0, 1), dsrc(dwt[s], 0, 1), Alu.mult)
                    nc.vector.tensor_tensor(
                        osrc(1, 7), tsrc(1, 7), dsrc(dwt[s], 0, 7), Alu.mult)
                    nc.vector.tensor_tensor(
                        osrc(0, 1), osrc(0, 1), dsrc(wt[s], 0, 1), Alu.add)
                    nc.vector.tensor_tensor(
                        osrc(1, 7), osrc(1, 7), dsrc(wt[s], 0, 7), Alu.add)
                elif kind == "U2":
                    def wsrc(nb, k):
                        return dsrc(wt[s], k, nb)

                    def msrc(b0, nb, k):
                        return apn(mts[k][:], b0 * 4 * A2T,
                                   (4 * A2T, nb), (A2T, 4), (1, A2T))

                    nc.vector.tensor_copy(osrc(0, 1), wsrc(1, 0))
                    nc.vector.tensor_copy(osrc(1, 7), wsrc(7, 0))
                    for k in range(3):
                        nc.vector.copy_predicated(
                            osrc(0, 1), msrc(0, 1, k), wsrc(1, k + 1))
                        nc.vector.copy_predicated(
                            osrc(1, 7), msrc(1, 7, k), wsrc(7, k + 1))
                else:  # B: pure copies, affine runs (d=0 dups use stride-0 src)
                    eng = (nc.scalar, nc.scalar, nc.scalar)[s - 5]
                    for (b0, L, src0, d) in _run_plan(offs[s]):
                        src0, d = (src0 - 1) // 2, d // 2  # odd-row packing
                        dst = apn(ot[s][:], b0 * A2T, (A2T, L), (1, A2T))
                        sap = apn(wt[s][:], src0 * A2T, (d * A2T, L), (1, A2T))
                        if eng is nc.scalar:
                            nc.scalar.copy(dst, sap)
                        else:
                            eng.tensor_copy(dst, sap)
                if mode != 'compute':
                    nc.scalar.dma_start(out_t[s], ot[s][:])
    nc.finalize()
    return nc


def _get_state():
    if not _STATE:
        starts, offs, wvec, tstream = _build_tables()
        _STATE["tables"] = (starts, offs, wvec, tstream)
        _STATE["nc"] = _build_nc(offs, wvec)
    return _STATE


def kernel(audio, crop_len=CL, start_pos=SP, **_):
    from concourse.bass_utils import run_bass_kernel_spmd
    from ml_dtypes import bfloat16

    audio = np.ascontiguousarray(np.asarray(audio), dtype=np.float32).reshape(-1)
    assert audio.shape[0] == N
    assert int(crop_len) == CL and int(start_pos) == SP

    st = _get_state()
    starts, offs, wvec, tstream = st["tables"]

    # pack per-core windows (cropped coords, clamped to last sample)
    pad = np.empty(CL + 64, dtype=np.float32)
    pad[:CL] = audio[SP:SP + CL]
    pad[CL:] = audio[SP + CL - 1]
    in_maps = [dict() for _ in range(8)]
    SLOT_ROWS = {"B": np.arange(1, 30, 2), "U1": np.arange(29),
                 "U2": np.arange(31), "A": np.arange(WROWS)}
    for s in range(8):
        rr = SLOT_ROWS[SLOT_KIND[s]]
        roff = (rr[:, None] + 28 * np.arange(A2T)[None, :]).reshape(-1)
        rows = starts[s]                          # [8, 128]
        gidx = rows.reshape(-1, 1) + roff[None, :]
        wins = pad[gidx].reshape(8, NPART, len(rr) * A2T).astype(bfloat16)
        for cid in range(8):
            in_maps[cid][f"win{s}"] = np.ascontiguousarray(wins[cid])
            if s in (0, 3):
                in_maps[cid][f"t{s}"] = np.ascontiguousarray(
                    tstream[s][cid].astype(bfloat16))
            elif s == 4:
                for k in range(3):
                    in_maps[cid][f"m4_{k + 1}"] = np.ascontiguousarray(
                        tstream[s][cid, k])
            if s in (1, 2):
                in_maps[cid][f"wv{s}"] = np.ascontiguousarray(
                    wvec[s][cid].astype(bfloat16))

    res = run_bass_kernel_spmd(st["nc"], in_maps, core_ids=list(range(8)))
    _STATE["last_results"] = res

    out = np.empty(N, dtype=np.float32)
    for s in range(8):
        for cid in range(8):
            tl = SLOT_TILES[s][cid]
            pm = np.asarray(res.results[cid]["out"][s]).astype(np.float32)
            pm = pm.reshape(NPART, 32, A2T)
            out[tl * TILE:(tl + 1) * TILE] = \
                pm.transpose(0, 2, 1).reshape(-1)
    return out


if __name__ == "__main__":
    rng = np.random.default_rng(0)
    audio = rng.standard_normal(N).astype(np.float32)
    got = kernel(audio, CL, SP)
    i = np.arange(N, dtype=np.int64)
    idx = (np.float32(0.875) * i.astype(np.float32)).astype(np.float32)
    idx[-1] = np.float32(CL)
    lo = np.floor(idx).astype(np.int64)
    hi = np.minimum(lo + 1, CL - 1)
    w = (idx - lo.astype(np.float32)).astype(np.float32)
    cropped = audio[SP:SP + CL]
    ref = ((np.float32(1.0) - w) * cropped[np.minimum(lo, CL - 1)]
           + w * cropped[hi]).astype(np.float32)
    err = np.abs(got - ref).max()
    print("max abs err vs numpy-ref:", err)



# revision 2
# speedup vs baseline: 1.3188x; 1.3188x over previous
"""Trainium2 Bass kernel for nn_RandomResizedCrop — v2, restructured.

Same phase-periodic decomposition as the baseline (64 output tiles of
524288 elems as 8 slots x 8 cores; see _build_tables), rebuilt around
three measurements:

  * the For_i back-edge is a full all-engine barrier (~2us), so the
    per-iteration time is the SINGLE-iteration critical path;
  * compute (54.7us) exceeded DMA (46.7us) in the baseline: DVE at
    0.96GHz, copy_predicated at 1x, and gpsimd tensor_tensor ~3x slower
    than DVE made the engine chain the binder;
  * t-streams / masks / wv are audio-INDEPENDENT constants.

Changes vs baseline:
  1. All metadata (t0/t3 streams, U2 masks, wv) is DMA'd ONCE before the
     loop and stays SBUF-resident: per-iteration HBM traffic drops from
     18.6MB to ~14.2MB per core (windows in + outputs out only).
  2. U2 select is inverted: base = w[row+1] (91% of elements), then
     predicated corrections only on planes where t==0 (4/32), t>=2
     (18/32), t>=3 (3/32) ever hold — 0.8 full passes of 1x cpred
     instead of 3.
  3. U2 base copy and two B slots run on ACT; one B slot and one A-slot
     dw run on gpsimd; DVE keeps U1/A/STT/cpred work. Input DMA order
     matches engine consumption order; output DMA triggers are ordered
     by expected completion to avoid ring head-of-line blocking.
  4. A-slot windows trimmed 32 -> 29 rows.
"""

import numpy as np

N = 33554432
CL = 29360128
SP = 1000000
TILE = 524288          # outputs per tile
FO = 4096              # outputs per partition
NPART = 128
A2T = FO // 32         # 128 phase blocks per partition
WROWS = 32

SLOT_TILES = [
    [0, 1, 2, 3, 5, 6, 7, 8],                # U1 (exact + oct-21 patterns)
    [10, 11, 12, 13, 14, 15, 16, 17],        # static-A oct-22
    [19, 20, 21, 22, 23, 24, 25, 26],        # static-A oct-23
    [4, 9, 18, 27, 28, 29, 30, 31],          # U1 (impure + oct-23 leftovers)
    [36, 32, 33, 34, 35, 61, 62, 63],        # U2 streamed integer select
    [37, 38, 39, 40, 41, 42, 43, 44],        # static-B oct-24
    [45, 46, 47, 48, 49, 50, 51, 52],        # static-B oct-24
    [53, 54, 55, 56, 57, 58, 59, 60],        # static-B oct-24
]
SLOT_KIND = ["U1", "A", "A", "U1", "U2", "B", "B", "B"]
SLOT_ROWS = {"B": 15, "U1": 29, "U2": 29, "A": 29}
# window groups: one DRAM tensor + one DMA per group.  Per-slot groups
# measured best (finer DMA granularity lets compute start earlier).
WIN_GROUPS = [(f"w{s}", (s,)) for s in range(8)]
WIN_ORDER = ("w4", "w1", "w0", "w3", "w2", "w5", "w6", "w7")
# software-pipelined pair schedule: body A tokens, body B tokens interleaved
# (W=window DMAs, g=gpsimd dw, u=U2 base+cpred, digits=slot phases)
PAIR_SCHED = "Wgu05 73Wgu 1620 5731 62".replace(" ", "")

E32 = (7 * np.arange(32)) // 8

_STATE = {}


def _run_plan(off):
    """Greedy segmentation of phases [0,32) into maximal affine runs.
    Returns list of (b0, L, src0, dstep): off[b0+r] == src0 + r*dstep."""
    plans, b = [], 0
    off = [int(x) for x in off]
    while b < 32:
        if b == 31:
            plans.append((b, 1, off[b], 0)); break
        d = off[b + 1] - off[b]
        L = 2
        while b + L < 32 and off[b + L] == off[b] + L * d:
            L += 1
        plans.append((b, L, off[b], d)); b += L
    return plans


def _j_merge(off, wv=None):
    """Check stride-8 (4-way) then stride-16 (2-way) phase merging."""
    off = [int(x) for x in off]
    for jb, nj in ((8, 4), (16, 2)):
        sstep = 7 * (jb // 8)
        ok = all(off[b + jb * j] == off[b] + sstep * j
                 for b in range(jb) for j in range(nj))
        if ok and wv is not None:
            ok = all(np.array_equal(wv[:, :, b + jb * j], wv[:, :, b])
                     for b in range(jb) for j in range(1, nj))
        if ok:
            return jb, nj, sstep
    return None


def _repack_j4(arr):
    """[.., 32, A2T] phase-major -> [.., 8, 4, A2T] op-major (b, j) order."""
    v = arr.reshape(arr.shape[:-1] + (32, A2T))
    v = v.reshape(v.shape[:-2] + (4, 8, A2T))   # [j, b, a]
    v = np.swapaxes(v, -3, -2)                  # [b, j, a]
    return np.ascontiguousarray(v).reshape(arr.shape)


def _plane_rects(bitmap):
    """bitmap[b, j] -> list of (b0, nb, j0, nj) rectangles (greedy by j,
    then runs over b) covering exactly the set bits."""
    rects = []
    for j in range(4):
        b = 0
        while b < 8:
            if bitmap[b, j]:
                b0 = b
                while b < 8 and bitmap[b, j]:
                    b += 1
                rects.append((b0, b - b0, j, 1))
            else:
                b += 1
    # merge identical (b0, nb) runs across adjacent j
    merged = []
    for r in sorted(rects, key=lambda r: (r[0], r[1], r[2])):
        if merged and merged[-1][0] == r[0] and merged[-1][1] == r[1] \
                and merged[-1][2] + merged[-1][3] == r[2]:
            merged[-1] = (r[0], r[1], merged[-1][2], merged[-1][3] + 1)
        else:
            merged.append(list(r))
            merged[-1] = tuple(merged[-1])
    return [tuple(m) for m in merged]


def _build_tables():
    i = np.arange(N, dtype=np.int64)
    idx = (np.float32(0.875) * i.astype(np.float32)).astype(np.float32)
    idx[-1] = np.float32(CL)
    a2 = i // 32
    b = i % 32
    T = idx.astype(np.float64) - (28 * a2 + E32[b])
    Tt = T.reshape(64, NPART, A2T, 32)

    starts, offs, wvec, tstream = {}, {}, {}, {}
    u2_planes = None
    for s, tiles in enumerate(SLOT_TILES):
        kind = SLOT_KIND[s]
        st = np.zeros((8, NPART), dtype=np.int64)
        if kind in ("A", "B"):
            off_ref = None
            wv = np.zeros((8, NPART, 32), dtype=np.float32)
            for c, tl in enumerate(tiles):
                Tp = Tt[tl]
                base = np.floor(Tp.min(axis=(1, 2))).astype(np.int64)
                trel = Tp - base[:, None, None]
                cmin = trel.min(axis=1)
                assert np.array_equal(cmin, trel.max(axis=1))
                off = np.floor(cmin).astype(np.int64)
                wv[c] = (cmin - off).astype(np.float32)
                assert np.all(off == off[0:1, :])
                if off_ref is None:
                    off_ref = off[0]
                assert np.array_equal(off[0], off_ref)
                if kind == "B":
                    assert np.all(wv[c] == 0)
                a2g = (tl * TILE + np.arange(NPART) * FO) // 32
                st[c] = 28 * a2g + base
            offs[s] = (E32 + off_ref).astype(np.int64)
            assert offs[s].max() <= WROWS - 2, offs[s].max()
            if kind == "A":
                assert offs[s].max() <= SLOT_ROWS["A"] - 2
            wvec[s] = wv
        elif kind == "U1":
            ts = np.zeros((8, NPART, 32, A2T), dtype=np.float32)
            for c, tl in enumerate(tiles):
                Tp = Tt[tl]
                base = np.floor(Tp.min(axis=(1, 2))).astype(np.int64)
                trel = Tp - base[:, None, None]
                t32 = trel.astype(np.float32)
                assert np.all(t32.astype(np.float64) == trel)
                assert 0 <= t32.min() and t32.max() <= 1.0
                ts[c] = np.transpose(t32, (0, 2, 1))
                a2g = (tl * TILE + np.arange(NPART) * FO) // 32
                st[c] = 28 * a2g + base
            tstream[s] = _repack_j4(ts.reshape(8, NPART, 32 * A2T))
        else:  # U2: integer t in {0..3} -> inverted-base masks
            ti_all = np.zeros((8, NPART, 32, A2T), dtype=np.int64)
            for c, tl in enumerate(tiles):
                Tp = Tt[tl]
                base = np.floor(Tp.min(axis=(1, 2))).astype(np.int64)
                trel = Tp - base[:, None, None]
                assert np.all(trel == np.round(trel)) and trel.max() <= 3.0
                ti_all[c] = np.transpose(trel.astype(np.int64), (0, 2, 1))
                a2g = (tl * TILE + np.arange(NPART) * FO) // 32
                st[c] = 28 * a2g + base
            # masks in repacked layout: m0 = (t==0), m2 = (t>=2), m3 = (t>=3)
            mk = np.zeros((8, 3, NPART, 32 * A2T), dtype=np.uint8)
            flat = ti_all.reshape(8, NPART, 32 * A2T)
            for c in range(8):
                mk[c, 0] = _repack_j4((flat[c] == 0).astype(np.uint8))
                mk[c, 1] = _repack_j4((flat[c] >= 2).astype(np.uint8))
                mk[c, 2] = _repack_j4((flat[c] >= 3).astype(np.uint8))
            tstream[s] = mk
            # active (b, j) planes per mask level (union over cores, p, a)
            v = mk.reshape(8, 3, NPART, 8, 4, A2T)
            u2_planes = [v[:, k].any(axis=(0, 1, 4)) for k in range(3)]
        starts[s] = st
    return starts, offs, wvec, tstream, u2_planes


def _build_nc(offs, wvec_chk=None, u2_planes=None, reps=1, mode='full',
              stag=False, s1_gp=False, unroll=1):
    import bass_rust
    import concourse.bacc as bacc
    import concourse.mybir as mybir
    from concourse.tile import TileContext

    bf16 = mybir.dt.bfloat16
    u8 = mybir.dt.uint8
    Alu = mybir.AluOpType

    nc = bacc.Bacc("TRN2", target_bir_lowering=False)
    t_t = {s: nc.dram_tensor(f"t{s}", [NPART, 32 * A2T], bf16,
                             kind="ExternalInput")
           for s in (0, 3)}
    m_t = [nc.dram_tensor(f"m4_{k}", [NPART, 32 * A2T], u8,
                          kind="ExternalInput") for k in range(3)]
    wv_t = {s: nc.dram_tensor(f"wv{s}", [NPART, 32], bf16,
                              kind="ExternalInput")
            for s in (1, 2)}
    rmaxs = {s: SLOT_ROWS[SLOT_KIND[s]] for s in range(8)}
    grp_rows = {nm: sum(rmaxs[s] for s in ss) for nm, ss in WIN_GROUPS}
    win_t = {nm: nc.dram_tensor(nm, [NPART, grp_rows[nm] * A2T], bf16,
                                kind="ExternalInput")
             for nm, _ in WIN_GROUPS}
    wgrp_of, woff = {}, {}
    for nm, ss in WIN_GROUPS:
        off = 0
        for s in ss:
            wgrp_of[s] = nm
            woff[s] = off
            off += rmaxs[s]
    out_t = nc.dram_tensor("out", [8, NPART, FO], bf16, kind="ExternalOutput")

    def apn(base, off, *dims):
        a = base.copy()
        part = list(a.ap[0])
        a.ap = bass_rust.VecI64Pair([part] + [list(d) for d in dims])
        a.offset = a.offset + off
        return a

    # B slots read odd rows 1..29, host packs as 15 rows
    for s in (5, 6, 7):
        used = sorted({src0 + r * d for (_, L, src0, d)
                       in _run_plan(offs[s]) for r in range(L)})
        assert used == list(range(1, 30, 2)), (s, used)

    ONLY = mode[4:] if mode.startswith('only') else None

    def on(s):
        return ONLY is None or SLOT_KIND[s] == ONLY

    do_compute = mode != 'dma'

    from contextlib import nullcontext
    with TileContext(nc) as tc:
        with tc.tile_pool(name="p", bufs=1) as pool:
            wg, ot, dwt, tt, wvt, mts = {}, {}, {}, {}, {}, []
            for nm, _ in WIN_GROUPS:
                wg[nm] = pool.tile([NPART, grp_rows[nm] * A2T], bf16,
                                   tag=nm, name=nm)
            for s in range(8):
                ot[s] = pool.tile([NPART, FO], bf16, tag=f"out{s}",
                                  name=f"ot{s}")
            for s in (0, 3):
                tt[s] = pool.tile([NPART, 32 * A2T], bf16, tag=f"t{s}",
                                  name=f"tt{s}")
            for k in range(3):
                mts.append(pool.tile([NPART, 32 * A2T], u8, tag=f"m{k}",
                                     name=f"mt{k}"))
            for s in (1, 2):
                wvt[s] = pool.tile([NPART, 32], bf16, tag=f"wv{s}",
                                   name=f"wvt{s}")
            for s in (0, 1, 2, 3):
                ndw = rmaxs[s] - 1
                dwt[s] = pool.tile([NPART, ndw * A2T], bf16,
                                   tag=f"dw{s}", name=f"dw{s}")

            # ---- constants: loaded once, SBUF-resident across the loop ----
            if do_compute:
                for s in (0, 3):
                    nc.sync.dma_start(tt[s][:], t_t[s][:])
                for k in range(3):
                    nc.sync.dma_start(mts[k][:], m_t[k][:])
                for s in (1, 2):
                    nc.sync.dma_start(wvt[s][:], wv_t[s][:])

            def wap(s, row0, *dims):
                """AP into slot s's window rows (inside its group blob)."""
                return apn(wg[wgrp_of[s]][:], (woff[s] + row0) * A2T, *dims)

            def emit_wins():
                # --- window blob DMAs in engine-consumption order ---
                # w4 first (feeds ACT base copy), then U1 pair (DVE),
                # A pair (gpsimd dw + DVE STT), B triple last.
                for nm in WIN_ORDER:
                    if any(on(s) for s in dict(WIN_GROUPS)[nm]):
                        if mode != 'compute':
                            nc.sync.dma_start(wg[nm][:], win_t[nm][:])
                        else:
                            nc.gpsimd.memset(wg[nm][:, :1], 0.0)

            if True:
                def osrc(s, b0, nb):
                    return apn(ot[s][:], b0 * A2T,
                               (A2T, nb), (8 * A2T, 4), (1, A2T))

                def dsrc(base_t, e0, nb, rstep=7):
                    return apn(base_t[:], e0 * A2T,
                               (A2T, nb), (rstep * A2T, 4), (1, A2T))

                def tsrc(s, b0, nb):
                    return apn(tt[s][:], b0 * 4 * A2T,
                               (4 * A2T, nb), (A2T, 4), (1, A2T))

                def wsub(s, row0, n):
                    """flat AP over n window rows starting at row0."""
                    return wap(s, row0, (1, n * A2T))

                def wdsrc(s, e0, nb, rstep=7):
                    return wap(s, e0, (A2T, nb), (rstep * A2T, 4), (1, A2T))

                def u1(s):
                    # dw = w[r+1] - w[r]; out = t*dw[E32-pattern] + w[...]
                    ndw = rmaxs[s] - 1
                    nc.vector.tensor_tensor(dwt[s][:], wsub(s, 1, ndw),
                                            wsub(s, 0, ndw), Alu.subtract)
                    nc.vector.tensor_tensor(
                        osrc(s, 0, 1), tsrc(s, 0, 1), dsrc(dwt[s], 0, 1),
                        Alu.mult)
                    nc.vector.tensor_tensor(
                        osrc(s, 1, 7), tsrc(s, 1, 7), dsrc(dwt[s], 0, 7),
                        Alu.mult)
                    nc.vector.tensor_tensor(
                        osrc(s, 0, 1), osrc(s, 0, 1), wdsrc(s, 0, 1),
                        Alu.add)
                    nc.vector.tensor_tensor(
                        osrc(s, 1, 7), osrc(s, 1, 7), wdsrc(s, 0, 7),
                        Alu.add)

                def a_stt(s):
                    jm = _j_merge(offs[s], wvec_chk[s])
                    assert jm is not None, (s, jm)
                    jb, nj, sstep = jm
                    for b in range(jb):
                        o = int(offs[s][b])
                        nc.vector.scalar_tensor_tensor(
                            apn(ot[s][:], b * A2T, (jb * A2T, nj), (1, A2T)),
                            apn(dwt[s][:], o * A2T, (sstep * A2T, nj),
                                (1, A2T)),
                            wvt[s][:, b:b + 1],
                            wap(s, o, (sstep * A2T, nj), (1, A2T)),
                            Alu.mult, Alu.add)

                def a_slot(s, dw_eng):
                    ndw = rmaxs[s] - 1
                    dw_eng.tensor_tensor(dwt[s][:], wsub(s, 1, ndw),
                                         wsub(s, 0, ndw), Alu.subtract)
                    a_stt(s)

                def u2(s):
                    # rows follow the E32 pattern: row(b,j) = E32[b] + 7j
                    # with E32[b] = max(b-1, 0) for b in [0,8).  The window
                    # is host-packed starting at original row 1 (rows 1..29),
                    # so packed row = original row - 1.
                    # base = w[row+1] on ACT, then plane-restricted cpreds:
                    #   m0: t==0 -> w[row+0], m2: t>=2 -> w[row+2],
                    #   m3: t>=3 -> w[row+3]
                    nc.scalar.copy(osrc(s, 0, 1), wdsrc(s, 0, 1))
                    nc.scalar.copy(osrc(s, 1, 7), wdsrc(s, 0, 7))
                    for k, roff in ((0, 0), (1, 2), (2, 3)):
                        for (b0, nb, j0, nj) in _plane_rects(u2_planes[k]):
                            segs = [(b0, nb)] if b0 >= 1 else \
                                ([(0, 1)] + ([(1, nb - 1)] if nb > 1 else []))
                            for (bb, nbb) in segs:
                                e0 = roff + (0 if bb == 0 else bb - 1) - 1
                                assert e0 + 7 * j0 >= 0, (k, bb, j0)
                                dst = apn(ot[s][:], (bb + 8 * j0) * A2T,
                                          (A2T, nbb), (8 * A2T, nj),
                                          (1, A2T))
                                msk = apn(mts[k][:], (bb * 4 + j0) * A2T,
                                          (4 * A2T, nbb), (A2T, nj),
                                          (1, A2T))
                                src = wap(s, e0 + 7 * j0,
                                          (A2T, nbb), (7 * A2T, nj),
                                          (1, A2T))
                                nc.vector.copy_predicated(dst, msk, src)

                def b_slot(s, eng):
                    for (b0, L, src0, d) in _run_plan(offs[s]):
                        src0, d = (src0 - 1) // 2, d // 2  # odd-row packing
                        dst = apn(ot[s][:], b0 * A2T, (A2T, L), (1, A2T))
                        sap = wap(s, src0, (d * A2T, L), (1, A2T))
                        if eng == 'act':
                            nc.scalar.copy(dst, sap)
                        else:
                            nc.gpsimd.tensor_copy(dst, sap)

                def out_dma(s):
                    if mode != 'compute' and on(s):
                        nc.scalar.dma_start(out_t[s], ot[s][:])

                def emit_dma_only_body():
                    emit_wins()
                    for s in range(8):
                        if on(s):
                            nc.gpsimd.memset(ot[s][:, :1], 0.0)
                    for s in (4, 0, 3, 1, 5, 2, 6, 7):
                        out_dma(s)

                def ph_gpdw():
                    if on(1):
                        ndw = rmaxs[1] - 1
                        nc.gpsimd.tensor_tensor(dwt[1][:], wsub(1, 1, ndw),
                                                wsub(1, 0, ndw),
                                                Alu.subtract)

                def ph_u2():
                    if on(4):
                        u2(4)          # ACT base copy + DVE cpreds
                    out_dma(4)

                def ph_s0():
                    if on(0):
                        u1(0)
                    out_dma(0)

                def ph_B5():
                    if on(5):
                        b_slot(5, 'act')
                    out_dma(5)

                def ph_B7():
                    if on(7):
                        b_slot(7, 'act')
                    out_dma(7)

                def ph_s3():
                    if on(3):
                        u1(3)
                    out_dma(3)

                def ph_s1():
                    if on(1):
                        a_stt(1)
                    out_dma(1)

                def ph_B6():
                    if on(6):
                        b_slot(6, 'gp')
                    out_dma(6)

                def ph_s2():
                    if on(2):
                        a_slot(2, nc.vector)
                    out_dma(2)

                # Emission order = per-engine ring order; HW sequencers
                # execute strictly in order with blocking sem waits, so each
                # trigger is placed where its producer should be done.
                def emit_body():
                    if not do_compute:
                        emit_dma_only_body()
                        return
                    emit_wins()
                    ph_gpdw(); ph_u2(); ph_s0(); ph_B5(); ph_B7(); ph_s3()
                    ph_s1(); ph_B6(); ph_s2()

                def emit_pair():
                    """Two bodies, software-pipelined: body B's windows and
                    early phases are interleaved into body A's stream so no
                    engine blocks on a not-yet-ready sem while later work is
                    ready.  Body B reuses the same tiles (bufs=1) — the Tile
                    scheduler inserts the cross-body deps."""
                    if not do_compute:
                        emit_dma_only_body()
                        emit_dma_only_body()
                        return
                    phs = {"W": emit_wins, "g": ph_gpdw, "u": ph_u2,
                           "0": ph_s0, "5": ph_B5, "7": ph_B7, "3": ph_s3,
                           "1": ph_s1, "6": ph_B6, "2": ph_s2}
                    for tok in PAIR_SCHED:
                        phs[tok]()

            if reps > 1:
                # Pair-pipelined loop: the back-edge is a full all-engine
                # barrier (~2us), so per-iteration = body latency unless
                # bodies inside the iteration overlap.
                lu = unroll if unroll > 1 else 1
                assert reps % lu == 0, (reps, lu)
                assert lu in (1, 2), lu
                with tc.For_i(0, reps // lu, 1, staggered_reset=stag):
                    if lu == 2:
                        emit_pair()
                    else:
                        emit_body()
            else:
                if unroll > 1 and unroll % 2 == 0:
                    for _ in range(unroll // 2):
                        emit_pair()
                else:
                    for _ in range(max(1, unroll)):
                        emit_body()
    nc.finalize()
    return nc


def _get_state():
    if not _STATE:
        starts, offs, wvec, tstream, u2_planes = _build_tables()
        _STATE["tables"] = (starts, offs, wvec, tstream, u2_planes)
        _STATE["nc"] = _build_nc(offs, wvec, u2_planes)
    return _STATE


def _rebuild(reps=1, mode='full', **kw):
    st = _get_state()
    _, offs, wvec, _, u2_planes = st["tables"]
    return _build_nc(offs, wvec, u2_planes, reps=reps, mode=mode, **kw)


def kernel(audio, crop_len=CL, start_pos=SP, **_):
    from concourse.bass_utils import run_bass_kernel_spmd
    from ml_dtypes import bfloat16

    audio = np.ascontiguousarray(np.asarray(audio),
                                 dtype=np.float32).reshape(-1)
    assert audio.shape[0] == N
    assert int(crop_len) == CL and int(start_pos) == SP

    st = _get_state()
    starts, offs, wvec, tstream, u2_planes = st["tables"]

    pad = np.empty(CL + 64, dtype=np.float32)
    pad[:CL] = audio[SP:SP + CL]
    pad[CL:] = audio[SP + CL - 1]
    in_maps = [dict() for _ in range(8)]
    ROWSETS = {"B": np.arange(1, 30, 2), "U1": np.arange(29),
               "U2": np.arange(1, 30), "A": np.arange(29)}
    wins = {}
    for s in range(8):
        rr = ROWSETS[SLOT_KIND[s]]
        roff = (rr[:, None] + 28 * np.arange(A2T)[None, :]).reshape(-1)
        rows = starts[s]                          # [8, 128]
        gidx = rows.reshape(-1, 1) + roff[None, :]
        wins[s] = pad[gidx].reshape(8, NPART, len(rr) * A2T).astype(bfloat16)
        for cid in range(8):
            if s in (0, 3):
                in_maps[cid][f"t{s}"] = np.ascontiguousarray(
                    tstream[s][cid].astype(bfloat16))
            elif s == 4:
                for k in range(3):
                    in_maps[cid][f"m4_{k}"] = np.ascontiguousarray(
                        tstream[s][cid, k])
            if s in (1, 2):
                in_maps[cid][f"wv{s}"] = np.ascontiguousarray(
                    wvec[s][cid].astype(bfloat16))
    for nm, ss in WIN_GROUPS:
        blob = np.concatenate([wins[s] for s in ss], axis=2)
        for cid in range(8):
            in_maps[cid][nm] = np.ascontiguousarray(blob[cid])

    res = run_bass_kernel_spmd(st["nc"], in_maps, core_ids=list(range(8)))
    _STATE["last_results"] = res

    out = np.empty(N, dtype=np.float32)
    for s in range(8):
        for cid in range(8):
            tl = SLOT_TILES[s][cid]
            pm = np.asarray(res.results[cid]["out"][s]).astype(np.float32)
            pm = pm.reshape(NPART, 32, A2T)
            out[tl * TILE:(tl + 1) * TILE] = \
                pm.transpose(0, 2, 1).reshape(-1)
    return out


if __name__ == "__main__":
    rng = np.random.default_rng(0)
    audio = rng.standard_normal(N).astype(np.float32)
    got = kernel(audio, CL, SP)
    i = np.arange(N, dtype=np.int64)
    idx = (np.float32(0.875) * i.astype(np.float32)).astype(np.float32)
    idx[-1] = np.float32(CL)
    lo = np.floor(idx).astype(np.int64)
    hi = np.minimum(lo + 1, CL - 1)
    w = (idx - lo.astype(np.float32)).astype(np.float32)
    cropped = audio[SP:SP + CL]
    ref = ((np.float32(1.0) - w) * cropped[np.minimum(lo, CL - 1)]
           + w * cropped[hi]).astype(np.float32)
    err = np.abs(got - ref).max()
    print("max abs err vs numpy-ref:", err)
